# revision 1
# baseline (speedup 1.0000x reference)
"""Cross-modal attention kernel for Trainium2 -- data-parallel over batch on 8 cores.

Reference computation per sample (C=256, H=W=64, N=H*W=4096, dqk=32):
    q = Wq @ x + bq; k = Wk @ y + bk; v = Wv @ y + bv
    out = gamma * (v @ softmax_j(q^T k)^T) + x

Strategy (per core = one batch sample):
  - Projections run in float32r, attention in bf16/fp8 so PE matmuls stream
    at 1 cycle/row (fp32 would be 4).
  - Energy is computed TRANSPOSED (E^T[j,i], keys on partitions) so the
    attention-weighted sum contracts over the partition dim with no
    transposes.  exp() is applied unnormalized (logits are O(1) by
    construction: gain-0.02 weights), softmax normalization happens on the
    [C, IBLK] output instead of the [N, N] matrix.
  - The K=32 energy matmuls are 4-way row-packed (tile_position).
  - exp(E^T) and v^T are stored fp8e4m3; AV and the denominator both run as
    MatmulPerfMode.DoubleRow contractions (2 fp8 weights/PE cell), pairing
    consecutive j-tiles via 3D [K,2,N] APs.  The denominator is a DoubleRow
    ones-matmul accumulating sum_j exp(E^T)[j,i] in PSUM.
  - Software pipelining: AV for group g-2 issues after the energy matmuls of
    group g; block n's normalization tail is deferred into block n+1.

Differences from the bf16 version:
  - exp(E^T) and v^T are stored as fp8e4m3; the AV contraction runs in
    MatmulPerfMode.DoubleRow (2 fp8 weights per PE cell -> half the cycles),
    pairing consecutive j-tiles along the partition dim via 3D [K,2,N] APs.
  - The softmax denominator is ALSO a DoubleRow matmul: ones[128,2,128] as
    stationary -> den[i] accumulates sum_j exp(E^T)[j,i] in PSUM, which
    removes the whole DVE accumulate+fold chain of the bf16 version.
  - gamma is applied as a per-partition tensor_scalar multiply on 1/den.
"""

import sys

if "/opt/trn_rl_repo" not in sys.path:
    sys.path.insert(0, "/opt/trn_rl_repo")

import numpy as np

import concourse.bacc as bacc
import concourse.mybir as mybir
import concourse.tile as tile
from concourse.bass_utils import run_bass_kernel_spmd

F32 = mybir.dt.float32
F32R = mybir.dt.float32r
BF16 = mybir.dt.bfloat16
FP8 = mybir.dt.float8e4

B, C, HW, D = 8, 256, 4096, 32
CH = C // 128
IBLK = 512
NIB = HW // IBLK
NJT = HW // 128
NPAIR = NJT // 2
EXPF = mybir.ActivationFunctionType.Exp
MULT = mybir.AluOpType.mult
ADD = mybir.AluOpType.add
DROW = mybir.MatmulPerfMode.DoubleRow


def _build():
    nc = bacc.Bacc("TRN2", target_bir_lowering=False, debug=False, num_devices=8)

    xr = nc.dram_tensor("xr", [C, HW], F32R, kind="ExternalInput")
    xf = nc.dram_tensor("xf", [C, HW], F32, kind="ExternalInput")
    yr = nc.dram_tensor("yr", [C, HW], F32R, kind="ExternalInput")
    wqT = nc.dram_tensor("wqT", [C, D], F32R, kind="ExternalInput")
    wkT = nc.dram_tensor("wkT", [C, D], F32R, kind="ExternalInput")
    wvT = nc.dram_tensor("wvT", [C, C], F32R, kind="ExternalInput")
    bqd = nc.dram_tensor("bqd", [D, 1], F32, kind="ExternalInput")
    bkd = nc.dram_tensor("bkd", [D, 1], F32, kind="ExternalInput")
    gbvd = nc.dram_tensor("gbvd", [128, CH], F32, kind="ExternalInput")
    gmd = nc.dram_tensor("gmd", [128, 1], F32, kind="ExternalInput")
    out = nc.dram_tensor("out", [C, HW], F32, kind="ExternalOutput")

    tc = tile.TileContext(nc)
    with tc:
        with (
            tc.tile_pool(name="cst", bufs=1) as cst,
            tc.tile_pool(name="qkv", bufs=1) as qkv,
        ):
            wq_sb = cst.tile([128, CH * D], F32R)
            wk_sb = cst.tile([128, CH * D], F32R)
            wv_sb = cst.tile([128, CH * C], F32R)
            bq_sb = cst.tile([D, 1], F32)
            bk_sb = cst.tile([D, 1], F32)
            gbv_sb = cst.tile([128, CH], F32)
            gm_sb = cst.tile([128, 1], F32)
            ones_sb = cst.tile([128, 2 * 128], FP8)
            nc.vector.memset(ones_sb[:], 1.0)
            nc.gpsimd.dma_start(bq_sb[:], bqd[:])
            nc.gpsimd.dma_start(bk_sb[:], bkd[:])
            nc.gpsimd.dma_start(gbv_sb[:], gbvd[:])
            nc.gpsimd.dma_start(gm_sb[:], gmd[:])

            q4 = qkv.tile([128, HW], BF16)
            k4 = qkv.tile([128, HW], BF16)
            vt = qkv.tile([128, NJT * C], FP8)

            NG = NJT // 4
            ptp = None  # assigned when the phase-B pools open
            psE = None

            def et_group(n, g, pt):
                # energy for (i-block n, group g): 4 row-packed K=32 matmuls
                # into two 2-bank psum tiles, then exp into pt (fp8)
                ets = [
                    psE.tile([128, 2 * IBLK], F32,
                             name=f"et{h}_{n}_{g}", tag="et", bufs=2)
                    for h in range(2)
                ]
                for q in range(4):
                    jt = 4 * g + q
                    nc.tensor.matmul(
                        ets[q // 2][:, (q % 2) * IBLK:(q % 2 + 1) * IBLK],
                        k4[32 * q:32 * (q + 1), jt * 128:(jt + 1) * 128],
                        q4[32 * q:32 * (q + 1), n * IBLK:(n + 1) * IBLK],
                        start=True,
                        stop=True,
                        tile_position=(32 * q, 0),
                    )
                for h in range(2):
                    nc.scalar.activation(
                        pt[:, (4 * g + 2 * h) * IBLK:(4 * g + 2 * h + 2) * IBLK],
                        ets[h][:], EXPF,
                    )

            with (
                tc.tile_pool(name="xy", bufs=1) as xy,
                tc.tile_pool(name="psA", bufs=4, space="PSUM") as psA,
            ):
                xr_sb = xy.tile([128, CH * HW], F32R)
                yr_sb = xy.tile([128, CH * HW], F32R)

                def in_chunk(src, dst_sb, h, c0, c1):
                    nc.sync.dma_start(
                        dst_sb[:, h * HW + c0: h * HW + c1],
                        src[h * 128:(h + 1) * 128, c0:c1],
                    )

                for h in range(CH):
                    nc.sync.dma_start(wq_sb[:, h * D:(h + 1) * D], wqT[h * 128:(h + 1) * 128, :])
                for h in range(CH):
                    in_chunk(xr, xr_sb, h, 0, IBLK)
                for h in range(CH):
                    nc.sync.dma_start(wk_sb[:, h * D:(h + 1) * D], wkT[h * 128:(h + 1) * 128, :])
                for h in range(CH):
                    in_chunk(yr, yr_sb, h, 0, IBLK)
                for h in range(CH):
                    nc.sync.dma_start(wv_sb[:, h * C:(h + 1) * C], wvT[h * 128:(h + 1) * 128, :])
                for ic in range(1, NIB):
                    c0, c1 = ic * IBLK, (ic + 1) * IBLK
                    for h in range(CH):
                        in_chunk(xr, xr_sb, h, c0, c1)
                        in_chunk(yr, yr_sb, h, c0, c1)
                for ic in range(NIB):
                    c0, c1 = ic * IBLK, (ic + 1) * IBLK
                    for w_sb, b_sb, src, dst in (
                        (wq_sb, bq_sb, xr_sb, q4),
                        (wk_sb, bk_sb, yr_sb, k4),
                    ):
                        ps = psA.tile([D, IBLK], F32, name=f"qk_{ic}", tag="qk_ps")
                        for h in range(CH):
                            nc.tensor.matmul(
                                ps[:],
                                w_sb[:, h * D:(h + 1) * D],
                                src[:, h * HW + c0: h * HW + c1],
                                start=(h == 0),
                                stop=(h == CH - 1),
                            )
                        nc.vector.tensor_scalar_add(
                            dst[0:D, c0:c1], ps[:], b_sb[:, 0:1]
                        )
                        for g in range(1, 4):
                            nc.gpsimd.dma_start(
                                dst[32 * g:32 * (g + 1), c0:c1], dst[0:D, c0:c1]
                            )
                    for jt in range(4 * ic, 4 * ic + 4):
                        ps = psA.tile([128, C], F32, name=f"vt_{jt}", tag="vt_ps")
                        for h in range(CH):
                            nc.tensor.matmul(
                                ps[:],
                                yr_sb[:, h * HW + jt * 128: h * HW + (jt + 1) * 128],
                                wv_sb[:, h * C:(h + 1) * C],
                                start=(h == 0),
                                stop=(h == CH - 1),
                            )
                        nc.vector.tensor_copy(vt[:, jt * C:(jt + 1) * C], ps[:])

            with (
                tc.tile_pool(name="ptp", bufs=2) as ptp,
                tc.tile_pool(name="wrk", bufs=2) as wrk,
                tc.tile_pool(name="psE", bufs=1, space="PSUM") as psE,
                tc.tile_pool(name="psAV", bufs=1, space="PSUM") as psAV,
            ):
                def make_tail(n, av, den):
                    def tail():
                        rgb = wrk.tile([128, IBLK], F32, name=f"rgb_{n}", tag="rgb")
                        nc.vector.reciprocal(rgb[:], den[:])
                        rgbg = wrk.tile([128, IBLK], F32, name=f"rgbg_{n}", tag="rgbg")
                        nc.vector.tensor_scalar(
                            rgbg[:], rgb[:], gm_sb[:, 0:1], None, MULT
                        )
                        for ch in range(CH):
                            xf_t = wrk.tile([128, IBLK], F32,
                                            name=f"xf_{n}_{ch}", tag="xf")
                            nc.sync.dma_start(
                                xf_t[:],
                                xf[ch * 128:(ch + 1) * 128, n * IBLK:(n + 1) * IBLK],
                            )
                            tmp = wrk.tile([128, IBLK], F32,
                                           name=f"tmp_{n}_{ch}", tag="tmp")
                            nc.vector.tensor_tensor(tmp[:], av[ch][:], rgbg[:], MULT)
                            ot = wrk.tile([128, IBLK], F32, name=f"ot_{n}_{ch}", tag="ot")
                            nc.vector.scalar_tensor_tensor(
                                ot[:], tmp[:], gbv_sb[:, ch:ch + 1], xf_t[:], ADD, ADD
                            )
                            nc.sync.dma_start(
                                out[ch * 128:(ch + 1) * 128, n * IBLK:(n + 1) * IBLK],
                                ot[:],
                            )
                    return tail

                ones_pair = ones_sb[:].rearrange("P (s c) -> P s c", s=2)

                pending_tail = None
                for n in range(NIB):
                    pt = ptp.tile([128, NJT * IBLK], FP8, name=f"pt_{n}", tag="pt")
                    av = [
                        psAV.tile([128, IBLK], F32, name=f"av{ch}_{n}", tag=f"av{ch}")
                        for ch in range(CH)
                    ]
                    den = psAV.tile([128, IBLK], F32, name=f"den_{n}", tag="den")

                    def av_pairs(g, pt=pt, av=av, den=den, n=n):
                        # DoubleRow AV + denominator for the 2 j-tile pairs of
                        # group g: virtual K=256 contracts two j-tiles at once
                        for p in (2 * g, 2 * g + 1):
                            ptp_ap = pt[:, 2 * p * IBLK:(2 * p + 2) * IBLK].rearrange(
                                "P (s N) -> P s N", s=2
                            )
                            vtp_ap = vt[:, 2 * p * C:(2 * p + 2) * C].rearrange(
                                "P (s c) -> P s c", s=2
                            )
                            for ch in range(CH):
                                nc.tensor.matmul(
                                    av[ch][:],
                                    vtp_ap[:, :, ch * 128:(ch + 1) * 128],
                                    ptp_ap,
                                    start=(p == 0),
                                    stop=(p == NPAIR - 1),
                                    perf_mode=DROW,
                                    skip_group_check=True,
                                )
                            nc.tensor.matmul(
                                den[:],
                                ones_pair,
                                ptp_ap,
                                start=(p == 0),
                                stop=(p == NPAIR - 1),
                                perf_mode=DROW,
                                skip_group_check=True,
                            )

                    for g in range(NG):
                        et_group(n, g, pt)
                        if g == 0 and pending_tail is not None:
                            pending_tail()
                            pending_tail = None
                        if g >= 2:
                            av_pairs(g - 2)
                    av_pairs(NG - 2)
                    av_pairs(NG - 1)
                    pending_tail = make_tail(n, av, den)
                pending_tail()
    nc.compile()
    return nc


_NC_CACHE = {}


def kernel(x, y, Wq, bq, Wk, bk, Wv, bv, gamma):
    assert x.shape == (B, C, 64, 64)
    xs = np.ascontiguousarray(x.reshape(B, C, HW).astype(np.float32))
    ys = np.ascontiguousarray(y.reshape(B, C, HW).astype(np.float32))
    wqT = np.ascontiguousarray(Wq.T.astype(np.float32))
    wkT = np.ascontiguousarray(Wk.T.astype(np.float32))
    wvT = np.ascontiguousarray(Wv.T.astype(np.float32))
    bqh = np.ascontiguousarray(bq.astype(np.float32).reshape(D, 1))
    bkh = np.ascontiguousarray(bk.astype(np.float32).reshape(D, 1))
    g = float(np.asarray(gamma).reshape(-1)[0])
    gbvh = np.ascontiguousarray((g * bv.astype(np.float32)).reshape(CH, 128).T)
    gmh = np.full((128, 1), g, dtype=np.float32)

    if "nc" not in _NC_CACHE:
        _NC_CACHE["nc"] = _build()
    nc = _NC_CACHE["nc"]

    in_maps = [
        {
            "xr": xs[b], "xf": xs[b], "yr": ys[b],
            "wqT": wqT, "wkT": wkT, "wvT": wvT,
            "bqd": bqh, "bkd": bkh, "gbvd": gbvh, "gmd": gmh,
        }
        for b in range(B)
    ]
    res = run_bass_kernel_spmd(nc, in_maps, list(range(B)))
    outs = np.stack([res.results[b]["out"] for b in range(B)])
    return outs.reshape(B, C, 64, 64).astype(np.float32)



# revision 15
# speedup vs baseline: 1.0588x; 1.0588x over previous
"""Cross-modal attention kernel for Trainium2 -- data-parallel over batch on 8 cores.

Reference computation per sample (C=256, H=W=64, N=H*W=4096, dqk=32):
    q = Wq @ x + bq; k = Wk @ y + bk; v = Wv @ y + bv
    out = gamma * (v @ softmax_j(q^T k)^T) + x

Design notes (cost-model driven):
  - Everything on the PE runs fp8e4m3 + MatmulPerfMode.DoubleRow (0.5
    cycles/output-row): the q/k/v projections contract C=256 as 128
    partitions x 2 (channel-chunk pairs), and the energy contracts the
    padded head dim 34 = 17 partitions x 2.  q/k live in a [17, 2, N]
    layout, which removes the old 4-way partition-broadcast entirely.
  - Weights are pre-scaled by S=16 on the host so their 0.02-sigma values
    sit in fp8's normal range; the scale is undone for free inside exp
    (scale=1/S^2 for q/k) and in the gamma tail scalar (gamma/S for v).
  - Biases are handled exactly: bv folds into the tail (+gamma*bv since
    softmax weights sum to 1); bq folds into an augmented k-row
    (S^2*bq@Wk) paired with a constant-1 q row; bk only shifts logits
    per-query, which softmax ignores.
  - exp of the [N, N] energy is split across THREE engines: ScalarE runs
    true exp (PSUM->fp8), while DVE and Pool run a Schraudolph fast-exp:
    one tensor_scalar computing round(a*E + b) written through an
    int8-bitcast AP directly into the fp8 tile (the int8 bits ARE the
    fp8 value; ~3.1% rms vs 2.7% rms for true-exp+fp8-quantize).
  - exp is applied unnormalized (logits are O(1)); normalization happens
    on the [C, 512] AV output.  The denominator is a DoubleRow
    ones-matmul accumulating sum_j pt[j,i] in PSUM.
  - All DMA is issued from the gpsimd (SWDGE, frees its sequencer
    immediately) or SP queues; inputs arrive as host-prequantized fp8
    (x8/y8) plus one f32 copy of x for the residual.  The old per-chunk
    HWDGE loads serialized ~173us of SP sequencer time; this issues 7
    large input DMAs instead.
  - Software pipelining: AV for unit u-2 issues after the energy+exp of
    unit u; block n's normalization tail and store are deferred into
    block n+1; q-projection for block n+1 is emitted mid-block n.
"""

import sys

if "/opt/trn_rl_repo" not in sys.path:
    sys.path.insert(0, "/opt/trn_rl_repo")

import numpy as np
import ml_dtypes

import concourse.bacc as bacc
import concourse.mybir as mybir
import concourse.tile as tile
from concourse.bass_utils import run_bass_kernel_spmd

F32 = mybir.dt.float32
FP8 = mybir.dt.float8e4
I8 = mybir.dt.int8
EXPF = mybir.ActivationFunctionType.Exp
MULT = mybir.AluOpType.mult
ADD = mybir.AluOpType.add
DROW = mybir.MatmulPerfMode.DoubleRow

B, C, HW, D = 8, 256, 4096, 32
CH = C // 128          # channel chunks
KP = 17                # q/k partitions (D padded to 34 = 17*2)
IBLK = 512             # i-block width
NIB = HW // IBLK       # 8 i-blocks
NJT = HW // 128        # 32 j-tiles
NU = NJT // 2          # 16 exp/AV units (2 j-tiles = 1024 cols) per block

WS = 16.0              # host weight pre-scale
ESCL = 1.0 / (WS * WS)  # undo q*k scale inside exp
A8 = 8.0 / np.log(2.0) * ESCL   # Schraudolph: bits = round(A8*E + B8)
B8 = 56.0 - 0.475

# exp-unit engine assignment per (block, unit): A=ScalarE, D=DVE.
# (gpsimd/Pool cannot read PSUM, so it cannot join the exp work.)
# Early blocks lean on ScalarE while DVE drains its phase-A convert backlog.
ASSIGN = (
    ["AAADAAAADAAADAAA"] * 2 + ["AADAADAADAADAADA"] * 6
)


def _build():
    nc = bacc.Bacc("TRN2", target_bir_lowering=False, debug=False, num_devices=8)

    x8d = nc.dram_tensor("x8d", [128, CH * HW], FP8, kind="ExternalInput")
    y8d = nc.dram_tensor("y8d", [128, CH * HW], FP8, kind="ExternalInput")
    xrd = nc.dram_tensor("xrd", [128, CH * HW], F32, kind="ExternalInput")
    wqk8d = nc.dram_tensor("wqk8d", [128, 4 * 64], FP8, kind="ExternalInput")
    wv8d = nc.dram_tensor("wv8d", [128, CH * C], FP8, kind="ExternalInput")
    # col 0: gamma/WS tail scalar; col 1: e15 q-bias row selector
    gmd = nc.dram_tensor("gmd", [128, 2], F32, kind="ExternalInput")
    out = nc.dram_tensor("out", [C, HW], F32, kind="ExternalOutput")

    tc = tile.TileContext(nc)
    with tc:
        with (
            tc.tile_pool(name="cst", bufs=1) as cst,
            tc.tile_pool(name="qkv", bufs=1) as qkv,
        ):
            wqk_sb = cst.tile([128, 4 * 64], FP8)
            wv_sb = cst.tile([128, CH * C], FP8)
            gm_sb = cst.tile([128, 2], F32)
            ones_sb = cst.tile([128, 2 * 128], FP8)
            nc.vector.memset(ones_sb[:], 1.0)

            x8 = qkv.tile([128, CH * HW], FP8)
            y8 = qkv.tile([128, CH * HW], FP8)
            xr = qkv.tile([128, CH * HW], F32)
            q8 = qkv.tile([KP, 2 * HW], FP8)
            k8 = qkv.tile([KP, 2 * HW], FP8)
            vt = qkv.tile([128, NJT * C], FP8)

            # DMAs: gpsimd (SWDGE) for what feeds the k/v path, SP for x.
            nc.gpsimd.dma_start(wqk_sb[:], wqk8d[:])
            nc.gpsimd.dma_start(wv_sb[:], wv8d[:])
            nc.gpsimd.dma_start(y8[:], y8d[:])
            nc.sync.dma_start(x8[:], x8d[:])
            nc.sync.dma_start(gm_sb[:], gmd[:])
            nc.sync.dma_start(xr[:], xrd[:])

            x8r = x8[:].rearrange("P (s n) -> P s n", s=2)
            y8r = y8[:].rearrange("P (s n) -> P s n", s=2)
            q8r = q8[:].rearrange("P (s n) -> P s n", s=2)
            k8r = k8[:].rearrange("P (s n) -> P s n", s=2)
            wvr = wv_sb[:].rearrange("P (s c) -> P s c", s=2)

            def wqk_ap(r):
                return wqk_sb[:, r * 64:(r + 1) * 64].rearrange(
                    "P (s c) -> P s c", s=2
                )[:, :, 0:KP]

            ptp = None
            psE = None
            psAV = None
            wrk = None
            qn = None

            def qk_proj(dst_r, src_r, r0, jb, ps_pool, tag, conv, nbufs=2):
                # projection of one 512-wide block of q or k: two DoubleRow
                # matmuls (s-halves of the padded head dim), then fp8 convert
                c0, c1 = jb * IBLK, (jb + 1) * IBLK
                for s in range(2):
                    ps = ps_pool.tile([KP, IBLK], F32,
                                      name=f"{tag}{jb}_{s}", tag=tag,
                                      bufs=nbufs)
                    nc.tensor.matmul(
                        ps[:], wqk_ap(r0 + s), src_r[:, :, c0:c1],
                        start=True, stop=True, perf_mode=DROW,
                        skip_group_check=True,
                    )
                    conv(dst_r[:, s:s + 1, c0:c1], ps[:], s)

            def kconv(dst, ps, s):
                nc.vector.tensor_copy(dst, ps[:])

            def qconv(dst, ps, s):
                if s == 0:
                    nc.vector.tensor_copy(dst, ps[:])
                else:
                    # adds the constant-1 augmented q row (partition 15)
                    nc.vector.tensor_scalar_add(dst, ps[:], gm_sb[0:KP, 1:2])

            with tc.tile_pool(name="psA", bufs=1, space="PSUM") as psA:
                for jb in range(NIB):
                    qk_proj(k8r, y8r, 2, jb, psA, "kps", kconv, nbufs=4)
                qk_proj(q8r, x8r, 0, 0, psA, "qps", qconv, nbufs=2)
                for jp in range(NJT // 2):
                    # two j-tiles of v share one [128, 2*C] psum tile; one
                    # fp8 convert for both
                    ps = psA.tile([128, 2 * C], F32, name=f"v{jp}",
                                  tag="vps", bufs=2)
                    for h in range(2):
                        nc.tensor.matmul(
                            ps[:, h * C:(h + 1) * C],
                            y8r[:, :, (2 * jp + h) * 128:(2 * jp + h + 1) * 128],
                            wvr,
                            start=True, stop=True, perf_mode=DROW,
                            skip_group_check=True,
                        )
                    nc.vector.tensor_copy(
                        vt[:, 2 * jp * C:(2 * jp + 2) * C], ps[:]
                    )

            with (
                tc.tile_pool(name="ptp", bufs=2) as ptp,
                tc.tile_pool(name="wrk", bufs=2) as wrk,
                tc.tile_pool(name="qn", bufs=1, space="PSUM") as qn,
                tc.tile_pool(name="psE", bufs=1, space="PSUM") as psE,
                tc.tile_pool(name="psAV", bufs=1, space="PSUM") as psAV,
            ):
                ones_pair = ones_sb[:].rearrange("P (s c) -> P s c", s=2)

                def make_tail(n, av, den):
                    def tail():
                        rgb = wrk.tile([128, IBLK], F32, name=f"rgb_{n}",
                                       tag="rgb")
                        nc.vector.reciprocal(rgb[:], den[:])
                        for ch in range(CH):
                            tmp = wrk.tile([128, IBLK], F32,
                                           name=f"tmp_{n}_{ch}", tag="tmp")
                            nc.vector.scalar_tensor_tensor(
                                tmp[:], av[ch][:], gm_sb[:, 0:1], rgb[:],
                                MULT, MULT,
                            )
                            ot = wrk.tile([128, IBLK], F32,
                                          name=f"ot_{n}_{ch}", tag="ot")
                            # xr already holds x + gamma*bv (host-folded)
                            nc.gpsimd.tensor_tensor(
                                ot[:], tmp[:],
                                xr[:, ch * HW + n * IBLK: ch * HW + (n + 1) * IBLK],
                                ADD,
                            )
                            nc.sync.dma_start(
                                out[ch * 128:(ch + 1) * 128,
                                    n * IBLK:(n + 1) * IBLK],
                                ot[:],
                            )
                    return tail

                pending_tail = None
                for n in range(NIB):
                    pt = ptp.tile([128, NJT * IBLK], FP8, name=f"pt_{n}",
                                  tag="pt")
                    ptr = pt[:].rearrange("P (u s n) -> P u s n", u=NU, s=2)
                    av = [
                        psAV.tile([128, IBLK], F32, name=f"av{ch}_{n}",
                                  tag=f"av{ch}")
                        for ch in range(CH)
                    ]
                    den = psAV.tile([128, IBLK], F32, name=f"den_{n}",
                                    tag="den")

                    def av_unit(u, pt=pt, av=av, den=den):
                        pt_ap = pt[:, 2 * u * IBLK:(2 * u + 2) * IBLK].rearrange(
                            "P (s n) -> P s n", s=2
                        )
                        vt_ap = vt[:, 2 * u * C:(2 * u + 2) * C].rearrange(
                            "P (s c) -> P s c", s=2
                        )
                        for ch in range(CH):
                            nc.tensor.matmul(
                                av[ch][:],
                                vt_ap[:, :, ch * 128:(ch + 1) * 128],
                                pt_ap,
                                start=(u == 0), stop=(u == NU - 1),
                                perf_mode=DROW, skip_group_check=True,
                            )
                        nc.tensor.matmul(
                            den[:], ones_pair, pt_ap,
                            start=(u == 0), stop=(u == NU - 1),
                            perf_mode=DROW, skip_group_check=True,
                        )

                    amap = ASSIGN[n]
                    for u in range(NU):
                        et = psE.tile([128, 2 * IBLK], F32,
                                      name=f"et_{n}_{u}", tag="et", bufs=2)
                        for h in range(2):
                            jt = 2 * u + h
                            nc.tensor.matmul(
                                et[:, h * IBLK:(h + 1) * IBLK],
                                k8r[:, :, jt * 128:(jt + 1) * 128],
                                q8r[:, :, n * IBLK:(n + 1) * IBLK],
                                start=True, stop=True, perf_mode=DROW,
                                skip_group_check=True,
                            )
                        dst = pt[:, 2 * u * IBLK:(2 * u + 2) * IBLK]
                        kind = amap[u]
                        if kind == "A":
                            nc.scalar.activation(dst, et[:], EXPF, scale=ESCL)
                        elif kind == "D":
                            nc.vector.tensor_scalar(
                                dst.bitcast(I8), et[:], A8, B8, MULT, ADD
                            )
                        else:
                            nc.gpsimd.tensor_scalar(
                                dst.bitcast(I8), et[:], A8, B8, MULT, ADD
                            )
                        if u == 0 and pending_tail is not None:
                            pending_tail()
                            pending_tail = None
                        if n < NIB - 1 and u in (5, 10):
                            s = 0 if u == 5 else 1
                            c0 = (n + 1) * IBLK
                            ps = qn.tile([KP, IBLK], F32,
                                         name=f"qn{n + 1}_{s}", tag="qn")
                            nc.tensor.matmul(
                                ps[:], wqk_ap(s),
                                x8r[:, :, c0:c0 + IBLK],
                                start=True, stop=True, perf_mode=DROW,
                                skip_group_check=True,
                            )
                            qconv(q8r[:, s:s + 1, c0:c0 + IBLK], ps, s)
                        if u >= 2:
                            av_unit(u - 2)
                    av_unit(NU - 2)
                    av_unit(NU - 1)
                    pending_tail = make_tail(n, av, den)
                pending_tail()
    nc.compile()
    return nc


_NC_CACHE = {}

F8NP = ml_dtypes.float8_e4m3


def _chunk128(a):
    # [256, HW] -> [128, 2*HW] with channel chunk as the s (free) dim
    return np.ascontiguousarray(
        a.reshape(CH, 128, -1).transpose(1, 0, 2).reshape(128, -1)
    )


def kernel(x, y, Wq, bq, Wk, bk, Wv, bv, gamma):
    assert x.shape == (B, C, 64, 64)
    xs = np.asarray(x, dtype=np.float32).reshape(B, C, HW)
    ys = np.asarray(y, dtype=np.float32).reshape(B, C, HW)
    Wq = np.asarray(Wq, dtype=np.float32)
    Wk = np.asarray(Wk, dtype=np.float32)
    Wv = np.asarray(Wv, dtype=np.float32)
    bq = np.asarray(bq, dtype=np.float32)
    g = float(np.asarray(gamma).reshape(-1)[0])

    # Augmented, pre-scaled q/k weights: rows 0-31 = WS*W, k-row 32 =
    # WS^2*bq@Wk (pairs with the constant-1 q row 32), row 33 = 0.
    Wqa = np.zeros((2 * KP, C), dtype=np.float32)
    Wqa[:D] = WS * Wq
    Wka = np.zeros((2 * KP, C), dtype=np.float32)
    Wka[:D] = WS * Wk
    Wka[D] = (WS * WS) * (bq @ Wk)
    # wqk8 layout: [p, r*64 + s*32 + m] = W_r[d'(m, half), s*128 + p]
    wqk8 = np.zeros((128, 4 * 64), dtype=np.float32)
    for r, (W, off) in enumerate(((Wqa, 0), (Wqa, KP), (Wka, 0), (Wka, KP))):
        blk = W[off:off + KP]  # [17, 256]
        for s in range(CH):
            wqk8[:, r * 64 + s * 32: r * 64 + s * 32 + KP] = (
                blk[:, s * 128:(s + 1) * 128].T
            )
    wv8 = _chunk128((WS * Wv).T)  # [p, s*256 + m] = WS*Wv[m, s*128+p]

    gmh = np.zeros((128, 2), dtype=np.float32)
    gmh[:, 0] = g / WS
    gmh[15, 1] = 1.0

    if "nc" not in _NC_CACHE:
        _NC_CACHE["nc"] = _build()
    nc = _NC_CACHE["nc"]

    # residual with gamma*bv folded in: out = gamma*(att@v0) + (x + gamma*bv)
    gbv = (g * np.asarray(bv, dtype=np.float32))[:, None]  # [C, 1]
    in_maps = []
    for b in range(B):
        xc = _chunk128(xs[b])
        in_maps.append({
            "x8d": xc.astype(F8NP),
            "y8d": _chunk128(ys[b]).astype(F8NP),
            "xrd": _chunk128(xs[b] + gbv),
            "wqk8d": wqk8.astype(F8NP),
            "wv8d": wv8.astype(F8NP),
            "gmd": gmh,
        })
    res = run_bass_kernel_spmd(nc, in_maps, list(range(B)))
    outs = np.stack([res.results[b]["out"] for b in range(B)])
    return outs.reshape(B, C, 64, 64).astype(np.float32)


# revision 59
# speedup vs baseline: 1.5006x; 1.4172x over previous
"""Cross-modal attention kernel for Trainium2 -- data-parallel over batch on 8 cores.

Reference computation per sample (C=256, H=W=64, N=H*W=4096, dqk=32):
    q = Wq @ x + bq; k = Wk @ y + bk; v = Wv @ y + bv
    out = gamma * (v @ softmax_j(q^T k)^T) + x

Design notes (cost-model driven):
  - Everything on the PE runs fp8e4m3 + MatmulPerfMode.DoubleRow (0.5
    cycles/output-row): the q/k/v projections contract C=256 as 128
    partitions x 2 (channel-chunk pairs), and the energy contracts the
    padded head dim 34 = 17 partitions x 2.  q/k live in a [17, 2, N]
    layout, which removes the old 4-way partition-broadcast entirely.
  - Weights are pre-scaled by S=16 on the host so their 0.02-sigma values
    sit in fp8's normal range; the scale is undone for free inside exp
    (scale=1/S^2 for q/k) and in the gamma tail scalar (gamma/S for v).
  - Biases are handled exactly: bv folds into the tail (+gamma*bv since
    softmax weights sum to 1); bq folds into an augmented k-row
    (S^2*bq@Wk) paired with a constant-1 q row; bk only shifts logits
    per-query, which softmax ignores.
  - exp of the [N, N] energy is split across THREE engines: ScalarE runs
    true exp (PSUM->fp8), while DVE and Pool run a Schraudolph fast-exp:
    one tensor_scalar computing round(a*E + b) written through an
    int8-bitcast AP directly into the fp8 tile (the int8 bits ARE the
    fp8 value; ~3.1% rms vs 2.7% rms for true-exp+fp8-quantize).
  - exp is applied unnormalized (logits are O(1)); normalization happens
    on the [C, 512] AV output.  The denominator is a DoubleRow
    ones-matmul accumulating sum_j pt[j,i] in PSUM.
  - All DMA is issued from the gpsimd (SWDGE, frees its sequencer
    immediately) or SP queues; inputs arrive as host-prequantized fp8
    (x8/y8) plus one f32 copy of x for the residual.  The old per-chunk
    HWDGE loads serialized ~173us of SP sequencer time; this issues 7
    large input DMAs instead.
  - Software pipelining: AV for unit u-2 issues after the energy+exp of
    unit u; block n's normalization tail and store are deferred into
    block n+1; q-projection for block n+1 is emitted mid-block n.
"""

import sys

if "/opt/trn_rl_repo" not in sys.path:
    sys.path.insert(0, "/opt/trn_rl_repo")

import numpy as np
import ml_dtypes

import concourse.bacc as bacc
import concourse.mybir as mybir
import concourse.tile as tile
from concourse.bass_utils import run_bass_kernel_spmd

F32 = mybir.dt.float32
FP8 = mybir.dt.float8e4
I8 = mybir.dt.int8
EXPF = mybir.ActivationFunctionType.Exp
MULT = mybir.AluOpType.mult
ADD = mybir.AluOpType.add
DROW = mybir.MatmulPerfMode.DoubleRow

B, C, HW, D = 8, 256, 4096, 32
CH = C // 128          # channel chunks
KP = 17                # q/k partitions (D padded to 34 = 17*2)
IBLK = 512             # i-block width
NIB = HW // IBLK       # 8 i-blocks
NJT = HW // 128        # 32 j-tiles
NU = NJT // 2          # 16 exp/AV units (2 j-tiles = 1024 cols) per block

WS = 16.0              # host weight pre-scale
ESCL = 1.0 / (WS * WS)  # undo q*k scale inside exp
A8 = 8.0 / np.log(2.0) * ESCL   # Schraudolph: bits = round(A8*E + B8)
B8 = 56.0 - 0.475

# exp-unit engine assignment per (block, j-tile): A=ScalarE, D=DVE.
# (gpsimd/Pool cannot read PSUM, so it cannot join the exp work.)
# ~11 of 32 units go to DVE; the last block shifts DVE work to its end so
# ScalarE drains early and the tail overlaps.
def _mk_assign(dpos):
    s = ["A"] * NU
    for p in dpos:
        s[p] = "D"
    return "".join(s)


# no two D adjacent (ring-3 hides one cross-engine hop, not two); keep the
# last two units on ScalarE so the trailing AV pairs aren't gated on DVE
import os as _os

_D_STEADY = [int(v) for v in _os.environ.get(
    "K_DPOS", "2,4,7,9,12,14").split(",")]
# last block: all DVE units early so the trailing AV/den pairs (gated on the
# final exps) depend only on ScalarE's in-stream work
_D_LAST = [int(v) for v in _os.environ.get(
    "K_DPOSL", "2,4,6,8,10").split(",")]
ASSIGN = [_mk_assign(_D_STEADY)] * 7 + [_mk_assign(_D_LAST)]
AV0_LAG = int(_os.environ.get("K_AV0LAG", "5"))
DEN_LAG = int(_os.environ.get("K_DENLAG", "5"))
FIN1_U = int(_os.environ.get("K_FIN1U", "3"))
AV1_BURST_U = int(_os.environ.get("K_AV1U", "3"))
LOOK = int(_os.environ.get("K_LOOK", "2"))


def _build():
    nc = bacc.Bacc("TRN2", target_bir_lowering=False, debug=False, num_devices=8)

    x8d = nc.dram_tensor("x8d", [128, CH * HW], FP8, kind="ExternalInput")
    y8d = nc.dram_tensor("y8d", [128, CH * HW], FP8, kind="ExternalInput")
    xrd = nc.dram_tensor("xrd", [128, CH * HW], F32, kind="ExternalInput")
    wqk8d = nc.dram_tensor("wqk8d", [128, 4 * 64], FP8, kind="ExternalInput")
    wv8d = nc.dram_tensor("wv8d", [128, CH * C], FP8, kind="ExternalInput")
    # col 0: gamma/WS tail scalar; col 1: e15 q-bias row selector
    gmd = nc.dram_tensor("gmd", [128, 2], F32, kind="ExternalInput")
    out = nc.dram_tensor("out", [C, HW], F32, kind="ExternalOutput")

    tc = tile.TileContext(nc)
    with tc:
        with (
            tc.tile_pool(name="cst", bufs=1) as cst,
            tc.tile_pool(name="qkv", bufs=1) as qkv,
        ):
            wqk_sb = cst.tile([128, 4 * 64], FP8)
            wv_sb = cst.tile([128, CH * C], FP8)
            gm_sb = cst.tile([128, 2], F32)
            ones_sb = cst.tile([128, 2 * 128], FP8)
            nc.vector.memset(ones_sb[:], 1.0)

            x8 = qkv.tile([128, CH * HW], FP8)
            y8 = qkv.tile([128, CH * HW], FP8)
            xr = qkv.tile([128, CH * HW], F32)
            q8 = qkv.tile([KP, 2 * HW], FP8)
            k8 = qkv.tile([KP, 2 * HW], FP8)
            vt = qkv.tile([128, NJT * C], FP8)

            # DMAs: gpsimd (SWDGE) for what feeds the k/v path, SP for x.
            # xr (the 4MB residual, 11.6us of DMA-engine time) must be LAST
            # in the shared DMA-engine queue -- it is only needed by the
            # first tail at ~20us, while y8/x8 gate all compute.
            # y8 in two column-halves so the first k/v projections start
            # ~1.5us earlier; tiny wqk goes via SP (otherwise idle) ahead of
            # y8 in the shared DMA-engine queue
            y8v = y8[:].rearrange("P (s n) -> P s n", s=2)
            y8dv = y8d[:].rearrange("P (s n) -> P s n", s=2)
            nc.gpsimd.dma_start(y8v[:, :, 0:HW // 2], y8dv[:, :, 0:HW // 2])
            nc.gpsimd.dma_start(y8v[:, :, HW // 2:HW], y8dv[:, :, HW // 2:HW])
            nc.gpsimd.dma_start(wv_sb[:], wv8d[:])
            nc.gpsimd.dma_start(x8[:], x8d[:])
            nc.gpsimd.dma_start(xr[:], xrd[:])
            nc.sync.dma_start(wqk_sb[:], wqk8d[:])
            nc.sync.dma_start(gm_sb[:], gmd[:])

            x8r = x8[:].rearrange("P (s n) -> P s n", s=2)
            y8r = y8[:].rearrange("P (s n) -> P s n", s=2)
            q8r = q8[:].rearrange("P (s n) -> P s n", s=2)
            k8r = k8[:].rearrange("P (s n) -> P s n", s=2)
            wvr = wv_sb[:].rearrange("P (s c) -> P s c", s=2)

            def wqk_ap(r):
                return wqk_sb[:, r * 64:(r + 1) * 64].rearrange(
                    "P (s c) -> P s c", s=2
                )[:, :, 0:KP]

            ptp = None
            psE = None
            psAV = None
            wrk = None
            qn = None

            def qk_proj(dst_r, src_r, r0, jb, ps_pool, tag, conv, nbufs=2):
                # projection of one 512-wide block of q or k: two DoubleRow
                # matmuls (s-halves of the padded head dim) into one 2-bank
                # psum tile, then a single fp8 convert
                c0, c1 = jb * IBLK, (jb + 1) * IBLK
                ps = ps_pool.tile([KP, 2 * IBLK], F32,
                                  name=f"{tag}{jb}", tag=tag, bufs=nbufs)
                for s in range(2):
                    nc.tensor.matmul(
                        ps[:, s * IBLK:(s + 1) * IBLK],
                        wqk_ap(r0 + s), src_r[:, :, c0:c1],
                        start=True, stop=True, perf_mode=DROW,
                        skip_group_check=True,
                    )
                conv(dst_r[:, :, c0:c1],
                     ps[:].rearrange("P (s n) -> P s n", s=2))

            def kconv(dst, ps, jb=0):
                # phase-A converts alternate Act/DVE so neither ring stalls PE
                if jb % 2 == 0:
                    nc.scalar.copy(dst, ps)
                else:
                    nc.vector.tensor_copy(dst, ps)

            def qconv(dst, ps):
                # q convert + the constant-1 augmented q row: the e15 bias
                # column only hits the s=1 half (partition 15 <-> d'=32)
                nc.vector.tensor_copy(dst[:, 0:1, :], ps[:, 0:1, :])
                nc.vector.tensor_scalar_add(
                    dst[:, 1:2, :], ps[:, 1:2, :], gm_sb[0:KP, 1:2]
                )

            def v_proj(jp, ps_pool):
                # two j-tiles of v share one [128, 2*C] psum tile; one
                # fp8 convert (DVE) for both
                ps = ps_pool.tile([128, 2 * C], F32, name=f"v{jp}",
                                  tag="vps", bufs=2)
                for h in range(2):
                    nc.tensor.matmul(
                        ps[:, h * C:(h + 1) * C],
                        y8r[:, :, (2 * jp + h) * 128:(2 * jp + h + 1) * 128],
                        wvr,
                        start=True, stop=True, perf_mode=DROW,
                        skip_group_check=True,
                    )
                if jp % 2 == 0:
                    nc.vector.tensor_copy(
                        vt[:, 2 * jp * C:(2 * jp + 2) * C], ps[:]
                    )
                else:
                    nc.scalar.copy(vt[:, 2 * jp * C:(2 * jp + 2) * C], ps[:])

            with tc.tile_pool(name="psA", bufs=1, space="PSUM") as psA:
                # interleave the Act-paced k stream with the DVE-paced v
                # stream so PE is never blocked on a single convert ring
                for jb in range(NIB):
                    qk_proj(k8r, y8r, 2, jb, psA, "kps",
                            lambda d, p, jb=jb: kconv(d, p, jb), nbufs=2)
                    v_proj(2 * jb, psA)
                    v_proj(2 * jb + 1, psA)
                qk_proj(q8r, x8r, 0, 0, psA, "qps", qconv, nbufs=1)

            with (
                tc.tile_pool(name="ptp", bufs=2) as ptp,
                tc.tile_pool(name="wrk", bufs=2) as wrk,
                tc.tile_pool(name="psE", bufs=1, space="PSUM") as psE,
                tc.tile_pool(name="psAV", bufs=1, space="PSUM") as psAV,
            ):
                ones_pair = ones_sb[:].rearrange("P (s c) -> P s c", s=2)

                def av_mm(acc_ap, ch, p, pt):
                    # one DoubleRow AV (ch=0/1) or denominator (ch=None)
                    # matmul for j-tile pair p, accumulating over all pairs
                    pt_ap = pt[:, 2 * p * IBLK:(2 * p + 2) * IBLK].rearrange(
                        "P (s n) -> P s n", s=2
                    )
                    if ch is None:
                        lhs = ones_pair
                    else:
                        lhs = vt[:, 2 * p * C:(2 * p + 2) * C].rearrange(
                            "P (s c) -> P s c", s=2
                        )[:, :, ch * 128:(ch + 1) * 128]
                    nc.tensor.matmul(
                        acc_ap, lhs, pt_ap,
                        start=(p == 0), stop=(p == NU - 1),
                        perf_mode=DROW, skip_group_check=True,
                    )

                def fin(n, ch, avt, rgb):
                    # normalize + residual + store for one channel chunk
                    tmp = wrk.tile([128, IBLK], F32,
                                   name=f"tmp_{n}_{ch}", tag="tmp")
                    nc.vector.scalar_tensor_tensor(
                        tmp[:], avt, gm_sb[:, 0:1], rgb[:], MULT, MULT
                    )
                    ot = wrk.tile([128, IBLK], F32,
                                  name=f"ot_{n}_{ch}", tag="ot")
                    # xr already holds x + gamma*bv (host-folded); the last
                    # block's ch1 add runs on DVE so it overlaps Pool's ch0
                    eng = nc.vector if (n == NIB - 1 and ch == 1) else nc.gpsimd
                    eng.tensor_tensor(
                        ot[:], tmp[:],
                        xr[:, ch * HW + n * IBLK: ch * HW + (n + 1) * IBLK],
                        ADD,
                    )
                    nc.sync.dma_start(
                        out[ch * 128:(ch + 1) * 128, n * IBLK:(n + 1) * IBLK],
                        ot[:],
                    )

                # av1 pass pacing inside the next block: pairs burst in two
                # halves at u==AV1_BURST_U and the next unit; den(n+1) shares
                # av1(n)'s bank, so stt1-ch1 (av1's reader, at FIN1_U) must
                # precede den's first accumulation (u==DEN_LAG).
                _AV1_PACE = [0] * NU
                _AV1_PACE[AV1_BURST_U] = NU // 2
                _AV1_PACE[AV1_BURST_U + 1] = NU - NU // 2

                def make_tails(n, pt, av0, den):
                    rgb = wrk.tile([128, IBLK], F32, name=f"rgb_{n}",
                                   tag="rgb")
                    if n == NIB - 1:
                        # last block: no next-block energies need the et ring,
                        # so av1 takes a free et slot and can start at
                        # pt-ready instead of after recip frees den's bank
                        av1_ap = psE.tile([128, 2 * IBLK], F32,
                                          name=f"av1_{n}", tag="et",
                                          bufs=3)[0:128, 0:IBLK]
                    else:
                        # second AV pass (ch1) reuses den's bank after recip
                        av1_ap = psAV.tile([128, IBLK], F32, name=f"av1_{n}",
                                           tag="den")[0:128, 0:IBLK]
                    st = {"p": 0}

                    def t1():
                        nc.vector.reciprocal(rgb[:], den[:])
                        fin(n, 0, av0[0:128, 0:IBLK], rgb)

                    def av1_step(k):
                        for _ in range(k):
                            if st["p"] < NU:
                                av_mm(av1_ap, 1, st["p"], pt)
                                st["p"] += 1

                    def fin1():
                        av1_step(NU)  # any remainder
                        fin(n, 1, av1_ap, rgb)

                    return t1, av1_step, fin1

                def emit_energy(n, u, et):
                    for h in range(2):
                        jt = 2 * u + h
                        nc.tensor.matmul(
                            et[:, h * IBLK:(h + 1) * IBLK],
                            k8r[:, :, jt * 128:(jt + 1) * 128],
                            q8r[:, :, n * IBLK:(n + 1) * IBLK],
                            start=True, stop=True, perf_mode=DROW,
                            skip_group_check=True,
                        )

                pending1 = pending_av1 = pending_fin1 = None
                pre_ets = []
                for n in range(NIB):
                    pt = ptp.tile([128, NJT * IBLK], FP8, name=f"pt_{n}",
                                  tag="pt")
                    av0 = psAV.tile([128, IBLK], F32, name=f"av0_{n}",
                                    tag="av")
                    den = psAV.tile([128, IBLK], F32, name=f"den_{n}",
                                    tag="den")
                    amap = ASSIGN[n]
                    blk_pre, pre_ets = pre_ets, []
                    for u in range(NU):
                        if u < len(blk_pre):
                            et = blk_pre[u]
                        else:
                            et = psE.tile([128, 2 * IBLK], F32,
                                          name=f"et_{n}_{u}", tag="et",
                                          bufs=3)
                            emit_energy(n, u, et)
                        dst = pt[:, 2 * u * IBLK:(2 * u + 2) * IBLK]
                        if amap[u] == "A":
                            nc.scalar.activation(dst, et[:], EXPF, scale=ESCL)
                        else:
                            nc.vector.tensor_scalar(
                                dst.bitcast(I8), et[:], A8, B8, MULT, ADD
                            )
                        if u == 0 and pending1 is not None:
                            pending1()
                            pending1 = None
                        if pending_av1 is not None:
                            pending_av1(_AV1_PACE[u])
                            if u == FIN1_U:
                                pending_fin1()
                                pending_av1 = pending_fin1 = None
                        if n < NIB - 1 and u in (5, 10):
                            # next block's q projection, in a spare et slot
                            s = 0 if u == 5 else 1
                            c0 = (n + 1) * IBLK
                            ps = psE.tile([128, 2 * IBLK], F32,
                                          name=f"qn{n + 1}_{s}", tag="et",
                                          bufs=3)
                            nc.tensor.matmul(
                                ps[0:KP, 0:IBLK], wqk_ap(s),
                                x8r[:, :, c0:c0 + IBLK],
                                start=True, stop=True, perf_mode=DROW,
                                skip_group_check=True,
                            )
                            dst_s = q8r[:, s:s + 1, c0:c0 + IBLK]
                            if s == 0:
                                nc.vector.tensor_copy(dst_s, ps[0:KP, 0:IBLK])
                            else:
                                nc.vector.tensor_scalar_add(
                                    dst_s, ps[0:KP, 0:IBLK], gm_sb[0:KP, 1:2]
                                )
                        if u >= AV0_LAG:
                            av_mm(av0[:], 0, u - AV0_LAG, pt)
                        if u >= DEN_LAG:
                            av_mm(den[:], None, u - DEN_LAG, pt)
                    if n < NIB - 1:
                        # lookahead: next block's first energies go ahead of
                        # the trailing AV pairs so ScalarE streams across the
                        # block boundary
                        for uu in range(LOOK):
                            t = psE.tile([128, 2 * IBLK], F32,
                                         name=f"et_{n + 1}_{uu}", tag="et",
                                         bufs=3)
                            emit_energy(n + 1, uu, t)
                            pre_ets.append(t)
                    for p in range(NU - AV0_LAG, NU):
                        av_mm(av0[:], 0, p, pt)
                    for p in range(NU - DEN_LAG, NU):
                        av_mm(den[:], None, p, pt)
                    pending1, pending_av1, pending_fin1 = make_tails(
                        n, pt, av0, den
                    )
                # final block: av1 runs in an et slot, gated only on pt --
                # emit it before the tail chain so it overlaps recip/fin0
                pending_av1(NU)
                pending1()
                pending_fin1()
    nc.compile()
    return nc


_NC_CACHE = {}

F8NP = ml_dtypes.float8_e4m3


def _chunk128(a):
    # [256, HW] -> [128, 2*HW] with channel chunk as the s (free) dim
    return np.ascontiguousarray(
        a.reshape(CH, 128, -1).transpose(1, 0, 2).reshape(128, -1)
    )


def kernel(x, y, Wq, bq, Wk, bk, Wv, bv, gamma):
    assert x.shape == (B, C, 64, 64)
    xs = np.asarray(x, dtype=np.float32).reshape(B, C, HW)
    ys = np.asarray(y, dtype=np.float32).reshape(B, C, HW)
    Wq = np.asarray(Wq, dtype=np.float32)
    Wk = np.asarray(Wk, dtype=np.float32)
    Wv = np.asarray(Wv, dtype=np.float32)
    bq = np.asarray(bq, dtype=np.float32)
    g = float(np.asarray(gamma).reshape(-1)[0])

    # Augmented, pre-scaled q/k weights: rows 0-31 = WS*W, k-row 32 =
    # WS^2*bq@Wk (pairs with the constant-1 q row 32), row 33 = 0.
    Wqa = np.zeros((2 * KP, C), dtype=np.float32)
    Wqa[:D] = WS * Wq
    Wka = np.zeros((2 * KP, C), dtype=np.float32)
    Wka[:D] = WS * Wk
    Wka[D] = (WS * WS) * (bq @ Wk)
    # wqk8 layout: [p, r*64 + s*32 + m] = W_r[d'(m, half), s*128 + p]
    wqk8 = np.zeros((128, 4 * 64), dtype=np.float32)
    for r, (W, off) in enumerate(((Wqa, 0), (Wqa, KP), (Wka, 0), (Wka, KP))):
        blk = W[off:off + KP]  # [17, 256]
        for s in range(CH):
            wqk8[:, r * 64 + s * 32: r * 64 + s * 32 + KP] = (
                blk[:, s * 128:(s + 1) * 128].T
            )
    wv8 = _chunk128((WS * Wv).T)  # [p, s*256 + m] = WS*Wv[m, s*128+p]

    gmh = np.zeros((128, 2), dtype=np.float32)
    gmh[:, 0] = g / WS
    gmh[15, 1] = 1.0

    if "nc" not in _NC_CACHE:
        _NC_CACHE["nc"] = _build()
    nc = _NC_CACHE["nc"]

    # residual with gamma*bv folded in: out = gamma*(att@v0) + (x + gamma*bv)
    gbv = (g * np.asarray(bv, dtype=np.float32))[:, None]  # [C, 1]
    in_maps = []
    for b in range(B):
        xc = _chunk128(xs[b])
        in_maps.append({
            "x8d": xc.astype(F8NP),
            "y8d": _chunk128(ys[b]).astype(F8NP),
            "xrd": _chunk128(xs[b] + gbv),
            "wqk8d": wqk8.astype(F8NP),
            "wv8d": wv8.astype(F8NP),
            "gmd": gmh,
        })
    res = run_bass_kernel_spmd(nc, in_maps, list(range(B)))
    outs = np.stack([res.results[b]["out"] for b in range(B)])
    return outs.reshape(B, C, 64, 64).astype(np.float32)


# revision 64
# speedup vs baseline: 1.5416x; 1.0273x over previous
"""Cross-modal attention kernel for Trainium2 -- data-parallel over batch on 8 cores.

Reference computation per sample (C=256, H=W=64, N=H*W=4096, dqk=32):
    q = Wq @ x + bq; k = Wk @ y + bk; v = Wv @ y + bv
    out = gamma * (v @ softmax_j(q^T k)^T) + x

Design notes (cost-model driven):
  - Everything on the PE runs fp8e4m3 + MatmulPerfMode.DoubleRow (0.5
    cycles/output-row): the q/k/v projections contract C=256 as 128
    partitions x 2 (channel-chunk pairs), and the energy contracts the
    padded head dim 34 = 17 partitions x 2.  q/k live in a [17, 2, N]
    layout, which removes the old 4-way partition-broadcast entirely.
  - Weights are pre-scaled by S=16 on the host so their 0.02-sigma values
    sit in fp8's normal range; the scale is undone for free inside exp
    (scale=1/S^2 for q/k) and in the gamma tail scalar (gamma/S for v).
  - Biases are handled exactly: bv folds into the tail (+gamma*bv since
    softmax weights sum to 1); bq folds into an augmented k-row
    (S^2*bq@Wk) paired with a constant-1 q row; bk only shifts logits
    per-query, which softmax ignores.
  - exp of the [N, N] energy is split across THREE engines: ScalarE runs
    true exp (PSUM->fp8), while DVE and Pool run a Schraudolph fast-exp:
    one tensor_scalar computing round(a*E + b) written through an
    int8-bitcast AP directly into the fp8 tile (the int8 bits ARE the
    fp8 value; ~3.1% rms vs 2.7% rms for true-exp+fp8-quantize).
  - exp is applied unnormalized (logits are O(1)); normalization happens
    on the [C, 512] AV output.  The denominator is a DoubleRow
    ones-matmul accumulating sum_j pt[j,i] in PSUM.
  - All DMA is issued from the gpsimd (SWDGE, frees its sequencer
    immediately) or SP queues; inputs arrive as host-prequantized fp8
    (x8/y8) plus one f32 copy of x for the residual.  The old per-chunk
    HWDGE loads serialized ~173us of SP sequencer time; this issues 7
    large input DMAs instead.
  - Software pipelining: AV for unit u-2 issues after the energy+exp of
    unit u; block n's normalization tail and store are deferred into
    block n+1; q-projection for block n+1 is emitted mid-block n.
"""

import sys

if "/opt/trn_rl_repo" not in sys.path:
    sys.path.insert(0, "/opt/trn_rl_repo")

import numpy as np
import ml_dtypes

import concourse.bacc as bacc
import concourse.mybir as mybir
import concourse.tile as tile
from concourse.bass_utils import run_bass_kernel_spmd

F32 = mybir.dt.float32
FP8 = mybir.dt.float8e4
I8 = mybir.dt.int8
EXPF = mybir.ActivationFunctionType.Exp
MULT = mybir.AluOpType.mult
ADD = mybir.AluOpType.add
DROW = mybir.MatmulPerfMode.DoubleRow

B, C, HW, D = 8, 256, 4096, 32
CH = C // 128          # channel chunks
KP = 17                # q/k partitions (D padded to 34 = 17*2)
IBLK = 512             # i-block width
NIB = HW // IBLK       # 8 i-blocks
NJT = HW // 128        # 32 j-tiles
NU = NJT // 2          # 16 exp/AV units (2 j-tiles = 1024 cols) per block

WS = 16.0              # host weight pre-scale
ESCL = 1.0 / (WS * WS)  # undo q*k scale inside exp
A8 = 8.0 / np.log(2.0) * ESCL   # Schraudolph: bits = round(A8*E + B8)
B8 = 56.0 - 0.475

# exp-unit engine assignment per (block, j-tile): A=ScalarE, D=DVE.
# (gpsimd/Pool cannot read PSUM, so it cannot join the exp work.)
# ~11 of 32 units go to DVE; the last block shifts DVE work to its end so
# ScalarE drains early and the tail overlaps.
def _mk_assign(dpos):
    s = ["A"] * NU
    for p in dpos:
        s[p] = "D"
    return "".join(s)


# no two D adjacent (ring-3 hides one cross-engine hop, not two); keep the
# last two units on ScalarE so the trailing AV pairs aren't gated on DVE
import os as _os

_D_STEADY = [int(v) for v in _os.environ.get(
    "K_DPOS", "2,4,7,9,12,14").split(",")]
# last block: all DVE units early so the trailing AV/den pairs (gated on the
# final exps) depend only on ScalarE's in-stream work
_D_LAST = [int(v) for v in _os.environ.get(
    "K_DPOSL", "2,4,7,9,12").split(",")]
ASSIGN = [_mk_assign(_D_STEADY)] * 7 + [_mk_assign(_D_LAST)]
AV0_LAG = int(_os.environ.get("K_AV0LAG", "5"))
DEN_LAG = int(_os.environ.get("K_DENLAG", "7"))
FIN1_U = int(_os.environ.get("K_FIN1U", "5"))
AV1_BURST_U = int(_os.environ.get("K_AV1U", "5"))
LOOK = int(_os.environ.get("K_LOOK", "2"))
Y8SPLIT = int(_os.environ.get("K_Y8SPLIT", "0"))
KPS3 = int(_os.environ.get("K_KPS3", "0"))


def _build():
    nc = bacc.Bacc("TRN2", target_bir_lowering=False, debug=False, num_devices=8)

    x8d = nc.dram_tensor("x8d", [128, CH * HW], FP8, kind="ExternalInput")
    y8d = nc.dram_tensor("y8d", [128, CH * HW], FP8, kind="ExternalInput")
    xrd = nc.dram_tensor("xrd", [128, CH * HW], F32, kind="ExternalInput")
    wqk8d = nc.dram_tensor("wqk8d", [128, 4 * 64], FP8, kind="ExternalInput")
    wv8d = nc.dram_tensor("wv8d", [128, CH * C], FP8, kind="ExternalInput")
    # col 0: gamma/WS tail scalar; col 1: e15 q-bias row selector
    gmd = nc.dram_tensor("gmd", [128, 2], F32, kind="ExternalInput")
    out = nc.dram_tensor("out", [C, HW], F32, kind="ExternalOutput")

    tc = tile.TileContext(nc)
    with tc:
        with (
            tc.tile_pool(name="cst", bufs=1) as cst,
            tc.tile_pool(name="qkv", bufs=1) as qkv,
        ):
            wqk_sb = cst.tile([128, 4 * 64], FP8)
            wv_sb = cst.tile([128, CH * C], FP8)
            gm_sb = cst.tile([128, 2], F32)
            ones_sb = cst.tile([128, 2 * 128], FP8)
            nc.vector.memset(ones_sb[:], 1.0)

            x8 = qkv.tile([128, CH * HW], FP8)
            y8 = qkv.tile([128, CH * HW], FP8)
            xr = qkv.tile([128, CH * HW], F32)
            q8 = qkv.tile([KP, 2 * HW], FP8)
            k8 = qkv.tile([KP, 2 * HW], FP8)
            vt = qkv.tile([128, NJT * C], FP8)

            # DMAs: gpsimd (SWDGE) for what feeds the k/v path, SP for x.
            # xr (the 4MB residual, 11.6us of DMA-engine time) must be LAST
            # in the shared DMA-engine queue -- it is only needed by the
            # first tail at ~20us, while y8/x8 gate all compute.
            # y8 in two column-halves so the first k/v projections start
            # ~1.5us earlier; tiny wqk goes via SP (otherwise idle) ahead of
            # y8 in the shared DMA-engine queue
            y8v = y8[:].rearrange("P (s n) -> P s n", s=2)
            y8dv = y8d[:].rearrange("P (s n) -> P s n", s=2)
            _yparts = ((0, 1024), (1024, 2048), (2048, HW)) if Y8SPLIT else \
                ((0, HW // 2), (HW // 2, HW))
            for c0, c1 in _yparts:
                nc.gpsimd.dma_start(y8v[:, :, c0:c1], y8dv[:, :, c0:c1])
            nc.gpsimd.dma_start(wv_sb[:], wv8d[:])
            nc.gpsimd.dma_start(x8[:], x8d[:])
            nc.gpsimd.dma_start(xr[:], xrd[:])
            nc.sync.dma_start(wqk_sb[:], wqk8d[:])
            nc.sync.dma_start(gm_sb[:], gmd[:])

            x8r = x8[:].rearrange("P (s n) -> P s n", s=2)
            y8r = y8[:].rearrange("P (s n) -> P s n", s=2)
            q8r = q8[:].rearrange("P (s n) -> P s n", s=2)
            k8r = k8[:].rearrange("P (s n) -> P s n", s=2)
            wvr = wv_sb[:].rearrange("P (s c) -> P s c", s=2)

            def wqk_ap(r):
                return wqk_sb[:, r * 64:(r + 1) * 64].rearrange(
                    "P (s c) -> P s c", s=2
                )[:, :, 0:KP]

            ptp = None
            psE = None
            psAV = None
            wrk = None
            qn = None

            def qk_proj(dst_r, src_r, r0, jb, ps_pool, tag, conv, nbufs=2):
                # projection of one 512-wide block of q or k: two DoubleRow
                # matmuls (s-halves of the padded head dim) into one 2-bank
                # psum tile, then a single fp8 convert
                c0, c1 = jb * IBLK, (jb + 1) * IBLK
                ps = ps_pool.tile([KP, 2 * IBLK], F32,
                                  name=f"{tag}{jb}", tag=tag, bufs=nbufs)
                for s in range(2):
                    nc.tensor.matmul(
                        ps[:, s * IBLK:(s + 1) * IBLK],
                        wqk_ap(r0 + s), src_r[:, :, c0:c1],
                        start=True, stop=True, perf_mode=DROW,
                        skip_group_check=True,
                    )
                conv(dst_r[:, :, c0:c1],
                     ps[:].rearrange("P (s n) -> P s n", s=2))

            def kconv(dst, ps, jb=0):
                # phase-A converts alternate Act/DVE so neither ring stalls PE
                if jb % 2 == 0:
                    nc.scalar.copy(dst, ps)
                else:
                    nc.vector.tensor_copy(dst, ps)

            def qconv(dst, ps):
                # q convert + the constant-1 augmented q row: the e15 bias
                # column only hits the s=1 half (partition 15 <-> d'=32)
                nc.vector.tensor_copy(dst[:, 0:1, :], ps[:, 0:1, :])
                nc.vector.tensor_scalar_add(
                    dst[:, 1:2, :], ps[:, 1:2, :], gm_sb[0:KP, 1:2]
                )

            def v_proj(jp, ps_pool):
                # two j-tiles of v share one [128, 2*C] psum tile; one
                # fp8 convert (DVE) for both
                ps = ps_pool.tile([128, 2 * C], F32, name=f"v{jp}",
                                  tag="vps", bufs=2)
                for h in range(2):
                    nc.tensor.matmul(
                        ps[:, h * C:(h + 1) * C],
                        y8r[:, :, (2 * jp + h) * 128:(2 * jp + h + 1) * 128],
                        wvr,
                        start=True, stop=True, perf_mode=DROW,
                        skip_group_check=True,
                    )
                if jp % 2 == 0:
                    nc.vector.tensor_copy(
                        vt[:, 2 * jp * C:(2 * jp + 2) * C], ps[:]
                    )
                else:
                    nc.scalar.copy(vt[:, 2 * jp * C:(2 * jp + 2) * C], ps[:])

            with tc.tile_pool(name="psA", bufs=1, space="PSUM") as psA:
                # interleave the Act-paced k stream with the DVE-paced v
                # stream so PE is never blocked on a single convert ring
                _kb = 3 if KPS3 else 2
                for jb in range(NIB):
                    qk_proj(k8r, y8r, 2, jb, psA, "kps",
                            lambda d, p, jb=jb: kconv(d, p, jb), nbufs=_kb)
                    v_proj(2 * jb, psA)
                    v_proj(2 * jb + 1, psA)
                # q0 shares the k ring (same tile shape) -- keeps psA at 8
                # banks while k gets a 3-deep ring
                if KPS3:
                    qk_proj(q8r, x8r, 0, 0, psA, "kps", qconv, nbufs=3)
                else:
                    qk_proj(q8r, x8r, 0, 0, psA, "qps", qconv, nbufs=1)

            with (
                tc.tile_pool(name="ptp", bufs=2) as ptp,
                tc.tile_pool(name="wrk", bufs=2) as wrk,
                tc.tile_pool(name="psE", bufs=1, space="PSUM") as psE,
                tc.tile_pool(name="psAV", bufs=1, space="PSUM") as psAV,
            ):
                ones_pair = ones_sb[:].rearrange("P (s c) -> P s c", s=2)

                def av_mm(acc_ap, ch, p, pt):
                    # one DoubleRow AV (ch=0/1) or denominator (ch=None)
                    # matmul for j-tile pair p, accumulating over all pairs
                    pt_ap = pt[:, 2 * p * IBLK:(2 * p + 2) * IBLK].rearrange(
                        "P (s n) -> P s n", s=2
                    )
                    if ch is None:
                        lhs = ones_pair
                    else:
                        lhs = vt[:, 2 * p * C:(2 * p + 2) * C].rearrange(
                            "P (s c) -> P s c", s=2
                        )[:, :, ch * 128:(ch + 1) * 128]
                    nc.tensor.matmul(
                        acc_ap, lhs, pt_ap,
                        start=(p == 0), stop=(p == NU - 1),
                        perf_mode=DROW, skip_group_check=True,
                    )

                def fin(n, ch, avt, rgb):
                    # normalize + residual + store for one channel chunk
                    tmp = wrk.tile([128, IBLK], F32,
                                   name=f"tmp_{n}_{ch}", tag="tmp")
                    nc.vector.scalar_tensor_tensor(
                        tmp[:], avt, gm_sb[:, 0:1], rgb[:], MULT, MULT
                    )
                    ot = wrk.tile([128, IBLK], F32,
                                  name=f"ot_{n}_{ch}", tag="ot")
                    # xr already holds x + gamma*bv (host-folded); the last
                    # block's ch1 add runs on DVE so it overlaps Pool's ch0
                    eng = nc.vector if (n == NIB - 1 and ch == 1) else nc.gpsimd
                    eng.tensor_tensor(
                        ot[:], tmp[:],
                        xr[:, ch * HW + n * IBLK: ch * HW + (n + 1) * IBLK],
                        ADD,
                    )
                    nc.sync.dma_start(
                        out[ch * 128:(ch + 1) * 128, n * IBLK:(n + 1) * IBLK],
                        ot[:],
                    )

                # av1 pass pacing inside the next block: pairs burst in two
                # halves at u==AV1_BURST_U and the next unit; den(n+1) shares
                # av1(n)'s bank, so stt1-ch1 (av1's reader, at FIN1_U) must
                # precede den's first accumulation (u==DEN_LAG).
                _AV1_PACE = [0] * NU
                _AV1_PACE[AV1_BURST_U] = NU // 2
                _AV1_PACE[AV1_BURST_U + 1] = NU - NU // 2

                def make_tails(n, pt, av0, den):
                    rgb = wrk.tile([128, IBLK], F32, name=f"rgb_{n}",
                                   tag="rgb")
                    if n == NIB - 1:
                        # last block: no next-block energies need the et ring,
                        # so av1 takes a free et slot and can start at
                        # pt-ready instead of after recip frees den's bank
                        av1_ap = psE.tile([128, 2 * IBLK], F32,
                                          name=f"av1_{n}", tag="et",
                                          bufs=3)[0:128, 0:IBLK]
                    else:
                        # second AV pass (ch1) reuses den's bank after recip
                        av1_ap = psAV.tile([128, IBLK], F32, name=f"av1_{n}",
                                           tag="den")[0:128, 0:IBLK]
                    st = {"p": 0}

                    def t1():
                        nc.vector.reciprocal(rgb[:], den[:])
                        fin(n, 0, av0[0:128, 0:IBLK], rgb)

                    def av1_step(k):
                        for _ in range(k):
                            if st["p"] < NU:
                                av_mm(av1_ap, 1, st["p"], pt)
                                st["p"] += 1

                    def fin1():
                        av1_step(NU)  # any remainder
                        fin(n, 1, av1_ap, rgb)

                    return t1, av1_step, fin1

                def emit_energy(n, u, et):
                    for h in range(2):
                        jt = 2 * u + h
                        nc.tensor.matmul(
                            et[:, h * IBLK:(h + 1) * IBLK],
                            k8r[:, :, jt * 128:(jt + 1) * 128],
                            q8r[:, :, n * IBLK:(n + 1) * IBLK],
                            start=True, stop=True, perf_mode=DROW,
                            skip_group_check=True,
                        )

                pending1 = pending_av1 = pending_fin1 = None
                pre_ets = []
                for n in range(NIB):
                    pt = ptp.tile([128, NJT * IBLK], FP8, name=f"pt_{n}",
                                  tag="pt")
                    av0 = psAV.tile([128, IBLK], F32, name=f"av0_{n}",
                                    tag="av")
                    den = psAV.tile([128, IBLK], F32, name=f"den_{n}",
                                    tag="den")
                    amap = ASSIGN[n]
                    blk_pre, pre_ets = pre_ets, []
                    for u in range(NU):
                        if u < len(blk_pre):
                            et = blk_pre[u]
                        else:
                            et = psE.tile([128, 2 * IBLK], F32,
                                          name=f"et_{n}_{u}", tag="et",
                                          bufs=3)
                            emit_energy(n, u, et)
                        dst = pt[:, 2 * u * IBLK:(2 * u + 2) * IBLK]
                        if amap[u] == "A":
                            nc.scalar.activation(dst, et[:], EXPF, scale=ESCL)
                        else:
                            nc.vector.tensor_scalar(
                                dst.bitcast(I8), et[:], A8, B8, MULT, ADD
                            )
                        if u == 0 and pending1 is not None:
                            pending1()
                            pending1 = None
                        if pending_av1 is not None:
                            pending_av1(_AV1_PACE[u])
                            if u == FIN1_U:
                                pending_fin1()
                                pending_av1 = pending_fin1 = None
                        if n < NIB - 1 and u in (5, 10):
                            # next block's q projection, in a spare et slot
                            s = 0 if u == 5 else 1
                            c0 = (n + 1) * IBLK
                            ps = psE.tile([128, 2 * IBLK], F32,
                                          name=f"qn{n + 1}_{s}", tag="et",
                                          bufs=3)
                            nc.tensor.matmul(
                                ps[0:KP, 0:IBLK], wqk_ap(s),
                                x8r[:, :, c0:c0 + IBLK],
                                start=True, stop=True, perf_mode=DROW,
                                skip_group_check=True,
                            )
                            dst_s = q8r[:, s:s + 1, c0:c0 + IBLK]
                            if s == 0:
                                nc.vector.tensor_copy(dst_s, ps[0:KP, 0:IBLK])
                            else:
                                nc.vector.tensor_scalar_add(
                                    dst_s, ps[0:KP, 0:IBLK], gm_sb[0:KP, 1:2]
                                )
                        if u >= AV0_LAG:
                            av_mm(av0[:], 0, u - AV0_LAG, pt)
                        if u >= DEN_LAG:
                            av_mm(den[:], None, u - DEN_LAG, pt)
                    if n < NIB - 1:
                        # lookahead: next block's first energies go ahead of
                        # the trailing AV pairs so ScalarE streams across the
                        # block boundary
                        for uu in range(LOOK):
                            t = psE.tile([128, 2 * IBLK], F32,
                                         name=f"et_{n + 1}_{uu}", tag="et",
                                         bufs=3)
                            emit_energy(n + 1, uu, t)
                            pre_ets.append(t)
                    for p in range(NU - AV0_LAG, NU):
                        av_mm(av0[:], 0, p, pt)
                    for p in range(NU - DEN_LAG, NU):
                        av_mm(den[:], None, p, pt)
                    pending1, pending_av1, pending_fin1 = make_tails(
                        n, pt, av0, den
                    )
                # final block: av1 runs in an et slot, gated only on pt --
                # emit it before the tail chain so it overlaps recip/fin0
                pending_av1(NU)
                pending1()
                pending_fin1()
    nc.compile()
    return nc


_NC_CACHE = {}

F8NP = ml_dtypes.float8_e4m3


def _chunk128(a):
    # [256, HW] -> [128, 2*HW] with channel chunk as the s (free) dim
    return np.ascontiguousarray(
        a.reshape(CH, 128, -1).transpose(1, 0, 2).reshape(128, -1)
    )


def kernel(x, y, Wq, bq, Wk, bk, Wv, bv, gamma):
    assert x.shape == (B, C, 64, 64)
    xs = np.asarray(x, dtype=np.float32).reshape(B, C, HW)
    ys = np.asarray(y, dtype=np.float32).reshape(B, C, HW)
    Wq = np.asarray(Wq, dtype=np.float32)
    Wk = np.asarray(Wk, dtype=np.float32)
    Wv = np.asarray(Wv, dtype=np.float32)
    bq = np.asarray(bq, dtype=np.float32)
    g = float(np.asarray(gamma).reshape(-1)[0])

    # Augmented, pre-scaled q/k weights: rows 0-31 = WS*W, k-row 32 =
    # WS^2*bq@Wk (pairs with the constant-1 q row 32), row 33 = 0.
    Wqa = np.zeros((2 * KP, C), dtype=np.float32)
    Wqa[:D] = WS * Wq
    Wka = np.zeros((2 * KP, C), dtype=np.float32)
    Wka[:D] = WS * Wk
    Wka[D] = (WS * WS) * (bq @ Wk)
    # wqk8 layout: [p, r*64 + s*32 + m] = W_r[d'(m, half), s*128 + p]
    wqk8 = np.zeros((128, 4 * 64), dtype=np.float32)
    for r, (W, off) in enumerate(((Wqa, 0), (Wqa, KP), (Wka, 0), (Wka, KP))):
        blk = W[off:off + KP]  # [17, 256]
        for s in range(CH):
            wqk8[:, r * 64 + s * 32: r * 64 + s * 32 + KP] = (
                blk[:, s * 128:(s + 1) * 128].T
            )
    wv8 = _chunk128((WS * Wv).T)  # [p, s*256 + m] = WS*Wv[m, s*128+p]

    gmh = np.zeros((128, 2), dtype=np.float32)
    gmh[:, 0] = g / WS
    gmh[15, 1] = 1.0

    if "nc" not in _NC_CACHE:
        _NC_CACHE["nc"] = _build()
    nc = _NC_CACHE["nc"]

    # residual with gamma*bv folded in: out = gamma*(att@v0) + (x + gamma*bv)
    gbv = (g * np.asarray(bv, dtype=np.float32))[:, None]  # [C, 1]
    in_maps = []
    for b in range(B):
        xc = _chunk128(xs[b])
        in_maps.append({
            "x8d": xc.astype(F8NP),
            "y8d": _chunk128(ys[b]).astype(F8NP),
            "xrd": _chunk128(xs[b] + gbv),
            "wqk8d": wqk8.astype(F8NP),
            "wv8d": wv8.astype(F8NP),
            "gmd": gmh,
        })
    res = run_bass_kernel_spmd(nc, in_maps, list(range(B)))
    outs = np.stack([res.results[b]["out"] for b in range(B)])
    return outs.reshape(B, C, 64, 64).astype(np.float32)
